# revision 16
# baseline (speedup 1.0000x reference)
"""Causal self-attention (B=2, T=2048, C=1024, 16 heads) on 8 trn2 NeuronCores.

Sharding: tensor-parallel, core c = b*4+g handles batch b (2) x head-group g
(4 heads = 256 channels). Each core computes q/k/v projections for its
channels, causal attention for its 4 heads, and the slice of the output
projection contracting its channels. Host sums the 4 partial outputs per
batch. No cross-core communication on device.
"""

import sys

if "/opt/trn_rl_repo" not in sys.path:
    sys.path.insert(0, "/opt/trn_rl_repo")

import numpy as np

import concourse.bass as bass
import concourse.mybir as mybir
from concourse.bass_utils import run_bass_kernel_spmd
from concourse.tile import TileContext
import concourse.tile_utils as _tile_utils

_tile_utils.max_sbuf_usage = 204 * 1024
from concourse.masks import make_identity
from concourse.vector_clock import ScopedClock

# ---------------------------------------------------------------------------
# Walrus on this image rejects >4 sem waits on a single instruction; the stock
# TileContext tail-drain attaches one wait per active logical processor.
# Split them into standalone wait_ge instructions instead.
def _patched_drain_and_barrier(self, tick_clock, wait_clock):
    probe = mybir.InstNoOp(name="wait_probe", ins=[], outs=[])
    probe.engine = mybir.EngineType.SP
    wait_clock.add_sem_waits(probe, ScopedClock({None: tick_clock.global_clock}))
    waits = (
        list(probe.sync_info.on_wait)
        if probe.sync_info and probe.sync_info.on_wait
        else []
    )
    assert self.sems is not None
    sem_by_num = {s.num: s for s in self.sems.allocated().values()}
    for w in waits:
        assert w.wait_mode == "sem-ge-imm", w
        self.nc.sync.wait_ge(sem_by_num[w.id], w.wait_value)
    self.nc.sync.drain()
    self.nc.all_engine_barrier()
    popped = self.nc._tile_sem_poison_stack.pop()
    assert popped is self._sem_poison
    self.nc.clear_and_free_semaphores(list(self.sems.allocated().values()))
    self.nc.all_engine_barrier()


TileContext._drain_and_barrier = _patched_drain_and_barrier

# The same walrus limit applies to regular instructions (matmul/LDWEIGHTS
# rejects even 2 waits). Split multi-wait instructions: excess waits move to
# single-wait NoOps committed just before on the same engine.
_orig_commit = TileContext._commit_instruction


def _split_commit(self, inst, lazy_reg_writes=True):
    si = inst.sync_info
    if (
        si is not None
        and si.on_wait
        and len(si.on_wait) > 1
        and inst.engine != mybir.EngineType.Unassigned
    ):
        waits = list(si.on_wait)
        for w in waits[:-1]:
            nop = mybir.InstNoOp(
                name=self.nc.get_next_instruction_name(),
                ins=[],
                outs=[],
                engine=inst.engine,
                sync_info=mybir.SyncInfo(on_wait=[w], on_update=[]),
                bass_nofuse=True,
            )
            _orig_commit(self, nop, lazy_reg_writes=False)
        inst.sync_info = mybir.SyncInfo(
            on_wait=[waits[-1]], on_update=list(si.on_update or [])
        )
    _orig_commit(self, inst, lazy_reg_writes)


TileContext._commit_instruction = _split_commit
# ---------------------------------------------------------------------------

N_CORES = 8
B, T, C = 2, 2048, 1024
H = 16
DH = C // H                       # 64
HPC = H // 4                      # 4 heads per core
CS = HPC * DH                     # 256 channels per core
SCALE = 1.0 / np.sqrt(np.float32(C))  # note: sqrt(n_embd), per reference

P = 128                           # partitions
TB = T // P                       # 16 t-blocks of 128
QC = T // 512                     # 4 q-chunks of 512
KO = C // P                       # 8 contraction subtiles for projections

F32 = mybir.dt.float32
BF16 = mybir.dt.bfloat16
# matmul compute dtype: float32r (tf32, full-rate PE) or float32 (exact, 4 cyc/row)
MM_DT = mybir.dt.float32r


def _bf16(a):
    import ml_dtypes
    return np.ascontiguousarray(a, dtype=np.float32).astype(ml_dtypes.bfloat16)


def _tf32_round(a):
    """Round-to-nearest-even fp32 -> tf32 (10-bit mantissa), returned as fp32 bits."""
    if MM_DT == F32:
        return np.ascontiguousarray(a, dtype=np.float32)
    u = np.ascontiguousarray(a, dtype=np.float32).view(np.uint32).astype(np.uint64)
    r = (u + 0x0FFF + ((u >> 13) & 1)) & 0xFFFFE000
    return r.astype(np.uint32).view(np.float32)

TRACE = False        # test.py flips this to profile
TRACE_KWARGS = {}
LAST_RESULT = None   # BassKernelResults of the most recent run

_NC_CACHE = None


def _build_nc():
    nc = bass.Bass()

    xT_d = nc.dram_tensor("xT", [C, T], MM_DT, kind="ExternalInput")
    wqT_d = nc.dram_tensor("wqT", [C, CS], BF16, kind="ExternalInput")
    wkT_d = nc.dram_tensor("wkT", [C, CS], BF16, kind="ExternalInput")
    wvT_d = nc.dram_tensor("wvT", [C, CS], MM_DT, kind="ExternalInput")
    xTb_d = nc.dram_tensor("xTb", [C, T], BF16, kind="ExternalInput")
    peT_d = nc.dram_tensor("peT", [CS, T], F32, kind="ExternalInput")
    woT_d = nc.dram_tensor("woT", [CS, C], MM_DT, kind="ExternalInput")
    maskT_d = nc.dram_tensor("maskT", [4, P, 512], F32, kind="ExternalInput")
    out_d = nc.dram_tensor("out", [T, C], F32, kind="ExternalOutput")

    with TileContext(nc) as tc:
        with (
            nc.allow_low_precision(reason="tf32 matmul inputs are rounded on purpose"),
            tc.tile_pool(name="const", bufs=1) as const,
            tc.tile_pool(name="xchunk", bufs=2) as xpool,
            tc.tile_pool(name="yu", bufs=6) as yupool,
            tc.tile_pool(name="pt", bufs=3) as ptpool,
            tc.tile_pool(name="rec", bufs=2) as recpool,
            tc.tile_pool(name="oddtmp", bufs=2) as oddpool,
            tc.tile_pool(name="outp", bufs=2) as outpool,
            tc.tile_pool(name="mm", bufs=2, space="PSUM") as mmps,
            tc.tile_pool(name="ypsum", bufs=2, space="PSUM") as yps,
            tc.tile_pool(name="bcpsum", bufs=2, space="PSUM") as bcps,
            tc.tile_pool(name="scratch", bufs=2, space="DRAM") as drampool,
        ):
            # ---- persistent tiles -------------------------------------------------
            wq_t = const.tile([P, KO, CS], BF16, tag="wq")
            wk_t = const.tile([P, KO, CS], BF16, tag="wk")
            wv_t = const.tile([P, KO, CS], MM_DT, tag="wv")
            wo_t = const.tile([P, 2, C], MM_DT, tag="wo")
            peT_t = const.tile([P, 2, T], F32, tag="peT")
            mask_t = const.tile([P, 4, 512], F32, tag="mask")
            qT_t = const.tile([P, 2, T], BF16, tag="qT")
            kT_t = const.tile([P, 2, T], BF16, tag="kT")
            v_t = const.tile([P, TB, HPC, DH + 1], MM_DT, tag="v")
            yTp_t = const.tile([P, 2, T], MM_DT, tag="yTp")
            ones_t = const.tile([1, DH], MM_DT, tag="ones")

            xT_r = xT_d.rearrange("(o p) t -> p o t", p=P)

            # DMA order = need order: wq/wk + x chunks gate the first matmuls;
            # mask/wo are not needed until attention / output projection.
            nc.sync.dma_start(out=wq_t[:], in_=wqT_d.rearrange("(o p) m -> p o m", p=P))
            nc.sync.dma_start(out=wk_t[:], in_=wkT_d.rearrange("(o p) m -> p o m", p=P))
            xTb_r = xTb_d.rearrange("(o p) t -> p o t", p=P)
            x_tiles = []
            xb_tiles = []
            for n in range(QC):
                xb_t = xpool.tile([P, KO, 512], BF16, tag="xb", name=f"xb_{n}")
                nc.sync.dma_start(
                    out=xb_t[:], in_=xTb_r[:, :, n * 512:(n + 1) * 512]
                )
                xb_tiles.append(xb_t)
                x_t = xpool.tile([P, KO, 512], MM_DT, tag="x", name=f"x_{n}")
                nc.sync.dma_start(
                    out=x_t[:], in_=xT_r[:, :, n * 512:(n + 1) * 512]
                )
                x_tiles.append(x_t)
                if n == 0:
                    nc.sync.dma_start(out=wv_t[:], in_=wvT_d.rearrange("(o p) m -> p o m", p=P))
                    nc.sync.dma_start(out=peT_t[:], in_=peT_d.rearrange("(o p) m -> p o m", p=P))
            nc.sync.dma_start(out=mask_t[:], in_=maskT_d.rearrange("d p m -> p d m"))
            ident_t = const.tile([P, P], F32, tag="ident")
            make_identity(nc, ident_t[:])
            ones_f32 = const.tile([P, TB * HPC], F32, tag="ones_f32")
            nc.gpsimd.memset(ones_f32[:], 1.0)
            nc.vector.tensor_copy(
                out=v_t[:, :, :, DH],
                in_=ones_f32.rearrange("p (a b) -> p a b", a=TB),
            )
            nc.vector.tensor_copy(out=ones_t[:], in_=ones_f32[:1, :DH])

            # ---- phase 1: q/k/v projections --------------------------------------
            for n in range(QC):
                ts = slice(n * 512, (n + 1) * 512)
                x_t = x_tiles[n]
                xb_t = xb_tiles[n]
                for (w_t, dst) in ((wq_t, qT_t), (wk_t, kT_t)):
                    for m in range(2):
                        ps_full = mmps.tile([P, 2, 512], F32, tag="mm")
                        ps = ps_full[:, 0, :]
                        for ko in range(KO):
                            nc.tensor.matmul(
                                ps,
                                lhsT=w_t[:, ko, m * P:(m + 1) * P],
                                rhs=xb_t[:, ko, :],
                                start=(ko == 0),
                                stop=(ko == KO - 1),
                            )
                        nc.any.tensor_add(
                            out=dst[:, m, ts], in0=ps, in1=peT_t[:, m, ts]
                        )
                for m in range(2):
                    psv_full = mmps.tile([P, 2, 512], F32, tag="mm")
                    psv = psv_full[:, 0, :]
                    for ko in range(KO):
                        nc.tensor.matmul(
                            psv,
                            lhsT=wv_t[:, ko, m * P:(m + 1) * P],
                            rhs=x_t[:, ko, :],
                            start=(ko == 0),
                            stop=(ko == KO - 1),
                        )
                    vtmp = oddpool.tile([P, 512], F32, tag="vtmp", name=f"vtmp_{n}_{m}")
                    nc.any.tensor_add(
                        out=vtmp[:], in0=psv, in1=peT_t[:, m, ts]
                    )
                    for tb4 in range(4):
                        tb = n * 4 + tb4
                        tr_ps = bcps.tile([P, P], F32, tag="bc", name=f"tr_{n}_{m}_{tb4}")
                        nc.tensor.transpose(
                            tr_ps[:], vtmp[:, tb4 * P:(tb4 + 1) * P], ident_t[:]
                        )
                        nc.vector.tensor_copy(
                            out=v_t[:, tb, 2 * m:2 * m + 2, :DH],
                            in_=tr_ps.rearrange("p (h d) -> p h d", h=2),
                        )

            # ---- phase 2: attention ----------------------------------------------
            # k-tiles processed in groups of 2; one exp covers both. In the
            # straddling groups (last four k-tiles of each q-chunk) the exp is
            # narrowed to the non-fully-masked columns and the 0/1 mask
            # multiply covers only columns below the full-valid region.
            for qc in range(QC):
                qs = slice(qc * 512, (qc + 1) * 512)
                yu_tiles = []
                for h in range(HPC):
                    hb = (h % 2) * DH
                    mt = h // 2
                    y_ps = yps.tile([DH + 1, 512], F32, tag="y")
                    nkt = 4 * qc + 4
                    for kg in range(nkt // 2):
                        st_ps = mmps.tile([P, 2, 512], F32, tag="mm")
                        pt_t = ptpool.tile([P, 2, 512], MM_DT, tag="pt")
                        for kcl in range(2):
                            kc = 2 * kg + kcl
                            nc.tensor.matmul(
                                st_ps[:, kcl, :],
                                lhsT=kT_t[hb:hb + DH, mt, kc * P:(kc + 1) * P],
                                rhs=qT_t[hb:hb + DH, mt, qs],
                                start=True,
                                stop=True,
                            )
                        d0 = 2 * kg - 4 * qc  # straddle index of first kc in group
                        if d0 < 0:
                            # fully-causal group: one wide exp, no mask
                            nc.scalar.activation(
                                pt_t.rearrange("p a b -> p (a b)"),
                                st_ps.rearrange("p a b -> p (a b)"),
                                mybir.ActivationFunctionType.Exp,
                                scale=float(SCALE),
                            )
                        else:
                            for kcl in range(2):
                                d = d0 + kcl
                                # columns < 128d are fully masked -> zero them via
                                # the mask product; exp only columns >= 128d
                                lo = 128 * d
                                nc.scalar.activation(
                                    pt_t[:, kcl, lo:],
                                    st_ps[:, kcl, lo:],
                                    mybir.ActivationFunctionType.Exp,
                                    scale=float(SCALE),
                                )
                                # multiply boundary band [lo, lo+128) by 0/1 mask
                                nc.any.tensor_mul(
                                    out=pt_t[:, kcl, lo:lo + P],
                                    in0=pt_t[:, kcl, lo:lo + P],
                                    in1=mask_t[:, d, lo:lo + P],
                                )
                                if d > 0:
                                    # zero the fully-masked columns
                                    nc.any.tensor_scalar_mul(
                                        pt_t[:, kcl, :lo], st_ps[:, kcl, :lo], 0.0
                                    )
                        for kcl in range(2):
                            kc = 2 * kg + kcl
                            nc.tensor.matmul(
                                y_ps[:],
                                lhsT=v_t[:, kc, h, :],
                                rhs=pt_t[:, kcl, :],
                                start=(kc == 0),
                                stop=(kc == nkt - 1),
                            )
                    # drain unnormalized y (+ sums row) to SBUF, freeing psum
                    yu = yupool.tile([DH + 1, 512], F32, tag="yu", name=f"yu_{qc}_{h}")
                    nc.vector.tensor_copy(out=yu[:], in_=y_ps[:])
                    yu_tiles.append(yu)

                # batched normalization for the 4 heads of this q-chunk:
                # gather sums rows into a [128, 16] layout via SBUF->SBUF DMA so
                # the (8-cycle-per-element) reciprocal runs on all lanes, then
                # scatter back to [1, 512] rows for the ones-broadcast matmul.
                sums_dram = drampool.tile([HPC, 512], F32, tag="sums_dram")
                for h in range(HPC):
                    nc.sync.dma_start(
                        out=sums_dram[h:h + 1, :], in_=yu_tiles[h][DH:DH + 1, :]
                    )
                s_resh = recpool.tile([P, HPC, 4], F32, tag="sresh")
                nc.sync.dma_start(
                    out=s_resh[:],
                    in_=sums_dram.rearrange("h (p j) -> p h j", p=P),
                )
                r_resh = recpool.tile([P, HPC, 4], MM_DT, tag="rresh")
                nc.vector.reciprocal(r_resh[:], s_resh[:])
                rec_dram = drampool.tile([HPC, 512], MM_DT, tag="rec_dram")
                nc.sync.dma_start(
                    out=rec_dram.rearrange("h (p j) -> p h j", p=P),
                    in_=r_resh[:],
                )
                rec_all = recpool.tile([1, HPC, 512], MM_DT, tag="recall")
                nc.sync.dma_start(out=rec_all[0:1, :, :], in_=rec_dram[None, :, :])
                for h in range(HPC):
                    mt = h // 2
                    bc_ps = bcps.tile([DH, 512], F32, tag="bc")
                    nc.tensor.matmul(
                        bc_ps[:], lhsT=ones_t[:], rhs=rec_all[0:1, h, :],
                        start=True, stop=True,
                    )
                    if h % 2 == 0:
                        nc.vector.tensor_mul(
                            out=yTp_t[:DH, mt, qs],
                            in0=yu_tiles[h][:DH, :], in1=bc_ps[:],
                        )
                    else:
                        ytn = oddpool.tile([DH, 512], MM_DT, tag="ytn")
                        nc.vector.tensor_mul(
                            out=ytn[:], in0=yu_tiles[h][:DH, :], in1=bc_ps[:]
                        )
                        # partition shift 0-63 -> 64-127 via SBUF->SBUF DMA
                        nc.sync.dma_start(out=yTp_t[DH:2 * DH, mt, qs], in_=ytn[:])

            # ---- phase 3: output projection --------------------------------------
            nc.sync.dma_start(out=wo_t[:], in_=woT_d.rearrange("(o p) m -> p o m", p=P))
            for tb in range(TB):
                tsl = slice(tb * P, (tb + 1) * P)
                o_t = outpool.tile([P, C], F32, tag="out")
                for oc in range(2):
                    ps_full = mmps.tile([P, 2, 512], F32, tag="mm")
                    ps = ps_full[:, 0, :]
                    for m in range(2):
                        nc.tensor.matmul(
                            ps,
                            lhsT=yTp_t[:, m, tsl],
                            rhs=wo_t[:, m, oc * 512:(oc + 1) * 512],
                            start=(m == 0),
                            stop=(m == 1),
                        )
                    nc.any.tensor_copy(
                        out=o_t[:, oc * 512:(oc + 1) * 512], in_=ps
                    )
                nc.sync.dma_start(out=out_d[tsl, :], in_=o_t[:])

    return nc


def _make_masks():
    kp = np.arange(P)[:, None]
    qf = np.arange(512)[None, :]
    m = np.empty((4, P, 512), dtype=np.float32)
    for d in range(4):
        m[d] = (kp <= qf - 128 * d).astype(np.float32)
    return m


def kernel(x, pos_emb, Wq, Wk, Wv, Wo):
    global _NC_CACHE, LAST_RESULT
    x = np.asarray(x, dtype=np.float32)
    pos_emb = np.asarray(pos_emb, dtype=np.float32)
    Wq = np.asarray(Wq, dtype=np.float32)
    Wk = np.asarray(Wk, dtype=np.float32)
    Wv = np.asarray(Wv, dtype=np.float32)
    Wo = np.asarray(Wo, dtype=np.float32)

    if _NC_CACHE is None:
        _NC_CACHE = _build_nc()
    nc = _NC_CACHE

    maskT = _make_masks()
    xT = [_tf32_round(x[b].T) for b in range(B)]
    xTb = [_bf16(x[b].T) for b in range(B)]
    in_maps = []
    for c in range(N_CORES):
        b, g = divmod(c, 4)
        ch = slice(g * CS, (g + 1) * CS)
        in_maps.append({
            "xT": xT[b],
            "wqT": _bf16(Wq[ch, :].T),
            "wkT": _bf16(Wk[ch, :].T),
            "wvT": _tf32_round(Wv[ch, :].T),
            "xTb": xTb[b],
            "peT": np.ascontiguousarray(pos_emb[:T, ch].T),
            "woT": _tf32_round(Wo[:, ch].T),
            "maskT": maskT,
        })

    res = run_bass_kernel_spmd(
        nc, in_maps, list(range(N_CORES)), trace=TRACE, **TRACE_KWARGS
    )
    LAST_RESULT = res

    out = np.zeros((B, T, C), dtype=np.float32)
    for c in range(N_CORES):
        b = c // 4
        out[b] += res.results[c]["out"]
    return out
